# revision 44
# baseline (speedup 1.0000x reference)
import sys, os
for _p in ("/opt/trn_rl_repo",):
    if _p not in sys.path:
        sys.path.append(_p)

import numpy as np
import ml_dtypes
from contextlib import ExitStack

import concourse.bass as bass
import concourse.bacc as bacc
import concourse.tile as tile
from concourse import mybir
from concourse.bass_utils import run_bass_kernel_spmd

F32 = mybir.dt.float32
BF16 = mybir.dt.bfloat16
BF_NP = ml_dtypes.bfloat16
FP8 = mybir.dt.float8e4
FP8_NP = ml_dtypes.float8_e4m3

DIM = 256
HEADS = 8
DIM_HEAD = 64
SLICE_NUM = 64
INNER = HEADS * DIM_HEAD  # 512
B, N = 4, 32768
NCORES = 8
NSHARD = N // 2  # 16384 tokens per core
P = 128
EXPF = mybir.ActivationFunctionType.Exp
DR = mybir.MatmulPerfMode.DoubleRow
ADD = mybir.AluOpType.add
MULT = mybir.AluOpType.mult
SUB = mybir.AluOpType.subtract
S_M2 = 256.0   # m2 fp8 range scale (folded into woT; undone in pass-2 epilogue)
M2_RESID = False  # fp8 error-feedback on m2 (2 extra pass-2 matmuls per tile)


def build_program(nshard, dbg=False):
    NT = nshard // P
    G = NT // 2            # 2-tile groups
    G_A = G_A_UNUSED = None  # single allreduce at end of pass 1
    GW = 2 * P             # tokens per group
    P2W = 512              # pass-2 tokens per psum tile
    NP2 = nshard // P2W
    nc = bacc.Bacc("TRN2", target_bir_lowering=False, debug=False,
                   num_devices=NCORES)
    if dbg:
        dbg_pooled = nc.dram_tensor("dbg_pooled", [P, 4, 257], F32,
                                    kind="ExternalOutput").ap()
        dbg_m2a = nc.dram_tensor("dbg_m2a", [P, 4, DIM], FP8,
                                 kind="ExternalOutput").ap()
        dbg_m2b = nc.dram_tensor("dbg_m2b", [P, 4, DIM], FP8,
                                 kind="ExternalOutput").ap()
        dbg_wT = nc.dram_tensor("dbg_wT", [P, 4, nshard], FP8,
                                kind="ExternalOutput").ap()
    x8T_h = nc.dram_tensor("x8T", [DIM, nshard], FP8, kind="ExternalInput")
    x8N_h = nc.dram_tensor("x8N", [nshard, DIM], FP8, kind="ExternalInput")
    w8T = nc.dram_tensor("w8T", [DIM, INNER], FP8, kind="ExternalInput")
    w816T = nc.dram_tensor("w816T", [DIM, INNER], FP8, kind="ExternalInput")
    id8x_h = nc.dram_tensor("id8x", [P, 2, GW], FP8, kind="ExternalInput").ap()
    bias8_h = nc.dram_tensor("bias8", [P, 2, P], FP8, kind="ExternalInput").ap()
    wfxT_h = nc.dram_tensor("wfxT", [P, 2, HEADS, 64], F32,
                            kind="ExternalInput").ap()
    bfxT_h = nc.dram_tensor("bfxT", [64, 4, 2], F32, kind="ExternalInput").ap()
    wqT = nc.dram_tensor("wqT", [64, 64], F32, kind="ExternalInput").ap()
    wkT = nc.dram_tensor("wkT", [64, 64], F32, kind="ExternalInput").ap()
    wvT = nc.dram_tensor("wvT", [64, 64], F32, kind="ExternalInput").ap()
    woT = nc.dram_tensor("woT", [64, HEADS, DIM], F32, kind="ExternalInput").ap()
    idf32 = nc.dram_tensor("idf32", [P, P], F32, kind="ExternalInput").ap()
    out_ap = nc.dram_tensor("outT", [DIM, nshard], BF16, kind="ExternalOutput").ap()

    with tile.TileContext(nc) as tc, ExitStack() as ctx:
        cpool = ctx.enter_context(tc.tile_pool(name="consts", bufs=1))
        big = ctx.enter_context(tc.tile_pool(name="big", bufs=1))
        x8pool = ctx.enter_context(tc.tile_pool(name="x8p", bufs=4))

        def x_group_load(g):
            x8t = x8pool.tile([P, 2, GW], FP8)
            nc.sync.dma_start(x8t[:], bass.AP(x8T_h, g * GW,
                              [[nshard, P], [P * nshard, 2], [1, GW]]))
            xnt = xn_ring[g % 4]
            nc.sync.dma_start(xnt[:, :, 0:DIM],
                              bass.AP(x8N_h, g * GW * DIM,
                                      [[DIM, P], [P * DIM, 2], [1, DIM]]))
            return x8t, xnt

        # logits weights first (longest pole), then x group 0
        w8_sb = cpool.tile([P, 2, INNER], FP8)
        nc.sync.dma_start(w8_sb[:], bass.AP(w8T, 0,
                          [[INNER, P], [P * INNER, 2], [1, INNER]]))
        w816_sb = cpool.tile([P, 2, INNER], FP8)
        nc.sync.dma_start(w816_sb[:], bass.AP(w816T, 0,
                          [[INNER, P], [P * INNER, 2], [1, INNER]]))
        # constant bias carrier: contributes rows 254/255 of w816T (= blg
        # fp8 + residual/16) uniformly to every token's logits
        bias8_sb = cpool.tile([P, 2, P], FP8)
        nc.sync.dma_start(bias8_sb[:], bias8_h[:])
        # natural-layout x tiles: ring of 3 with the norm ones-column preset
        xn_ring = [big.tile([P, 2, DIM + 1], FP8, name=f"xn{i}")
                   for i in range(4)]
        for i in range(4):
            nc.vector.memset(xn_ring[i][:, :, DIM:DIM + 1], 1.0)
        g0 = x_group_load(0)
        id8x_sb = cpool.tile([P, 2, GW], FP8)
        nc.gpsimd.dma_start(id8x_sb[:], id8x_h[:])
        # attention/pass-2 consts on the gpsimd queue
        wfxT_sb = cpool.tile([P, 2, HEADS, 64], F32)
        nc.gpsimd.dma_start(wfxT_sb[:], wfxT_h[:])
        bfxT_sb = cpool.tile([64, 4, 2], F32)
        nc.gpsimd.dma_start(bfxT_sb[:], bfxT_h[:])
        wq_sb = cpool.tile([64, 64], F32)
        wk_sb = cpool.tile([64, 64], F32)
        wv_sb = cpool.tile([64, 64], F32)
        nc.gpsimd.dma_start(wq_sb[:], wqT[:])
        nc.gpsimd.dma_start(wk_sb[:], wkT[:])
        nc.gpsimd.dma_start(wv_sb[:], wvT[:])
        wo_sb = cpool.tile([64, HEADS, DIM], F32)
        nc.gpsimd.dma_start(wo_sb[:], woT[:])
        idf_sb = cpool.tile([P, P], F32)
        nc.gpsimd.dma_start(idf_sb[:], idf32[:])

        # persistent across phases
        wT_sb = big.tile([P, 4, nshard], FP8)    # transposed normalized w
        pooled_sb = big.tile([P, 4, 257], BF16)  # allreduced pooled sums
        m28a_sb = big.tile([P, 4, DIM], FP8)
        m28b_sb = big.tile([P, 4, DIM], FP8)

        # ---------------- pass 1 ----------------
        with tc.tile_pool(name="sp", bufs=6) as spool, \
             tc.tile_pool(name="lgps", bufs=1, space="PSUM") as lgps, \
             tc.tile_pool(name="tps", bufs=1, space="PSUM") as tps, \
             tc.tile_pool(name="poolps", bufs=1, space="PSUM") as poolps, \
             tc.tile_pool(name="ccdram", bufs=1, space="DRAM") as dpool:
            pool_ps = [poolps.tile([P, DIM + 1], F32, name=f"pool_ps{i}")
                       for i in range(4)]
            ccA_in = dpool.tile([P, 4, 257], BF16)
            ccA_out = dpool.tile([P, 4, 257], BF16)

            # dummy collective to pay CC ucode/mesh setup off-critical-path
            wu_in = dpool.tile([1, 2], F32)
            wu_out = dpool.tile([1, 2], F32)
            wu_sb = big.tile([1, 2], F32)
            nc.gpsimd.memset(wu_sb[:], 0.0)
            nc.gpsimd.dma_start(wu_in[:], wu_sb[:])
            nc.gpsimd.collective_compute(
                "AllReduce", ADD,
                replica_groups=[[0, 1], [2, 3], [4, 5], [6, 7]],
                ins=[wu_in.opt()], outs=[wu_out.opt()])

            def emit_cc_copies():
                pre = big.tile([P, 4, 257], BF16, name="preA")
                with nc.allow_low_precision(reason="pooled sums fit bf16"):
                    for q in range(4):
                        nc.scalar.copy(pre[:, q, :], pool_ps[q][:])
                nc.sync.dma_start(ccA_in[:], pre[:])

            def emit_cc():
                nc.gpsimd.collective_compute(
                    "AllReduce", ADD,
                    replica_groups=[[0, 1], [2, 3], [4, 5], [6, 7]],
                    ins=[ccA_in.opt()], outs=[ccA_out.opt()])
                nc.sync.dma_start(pooled_sb[:], ccA_out[:])

            def pool_and_transpose(g, w2, xnt):
                for q in range(4):
                    nc.tensor.matmul(pool_ps[q][:],
                                     w2[:, :, 2 * q:2 * q + 2, :], xnt[:],
                                     start=(g == 0),
                                     stop=(g == G - 1),
                                     perf_mode=DR)
                if g == G - 1:
                    emit_cc_copies()
                wTp = tps.tile([P, 4, GW], F32)
                for c in range(4):
                    nc.tensor.matmul(wTp[:, c, :],
                                     w2[:, :, 2 * c:2 * c + 2, :],
                                     id8x_sb[:], start=True, stop=True,
                                     perf_mode=DR)
                with nc.allow_low_precision(reason="w is fp8 by design"):
                    nc.vector.tensor_copy(
                        wT_sb[:, 0:2, g * GW:(g + 1) * GW], wTp[:, 0:2, :])
                    nc.scalar.copy(
                        wT_sb[:, 2:4, g * GW:(g + 1) * GW], wTp[:, 2:4, :])

            prev = None
            for g in range(G):
                x8t, xnt = g0 if g == 0 else x_group_load(g)
                lgp2 = lgps.tile([P, 2, HEADS, SLICE_NUM], F32)
                for s in range(2):
                    xa = x8t[:, :, s * P:(s + 1) * P]
                    o = lgp2[:, s, :, :]
                    nc.tensor.matmul(o, xa, w8_sb[:], start=True, stop=False,
                                     perf_mode=DR)
                    nc.tensor.matmul(o, bias8_sb[:], w816_sb[:], start=False,
                                     stop=True, perf_mode=DR)
                e2 = spool.tile([P, 2, HEADS, SLICE_NUM], BF16)
                s2 = spool.tile([P, 2, HEADS], F32)
                r2 = spool.tile([P, 2, HEADS], F32)
                w2 = spool.tile([P, 2, HEADS, SLICE_NUM], FP8)
                # per-tile exp: releases each lgp bank as soon as it is read,
                # so the next group's logits matmuls overlap the second exp
                for par in range(2):
                    nc.scalar.activation(e2[:, par], lgp2[:, par], EXPF)
                    nc.vector.tensor_reduce(s2[:, par], e2[:, par],
                                            axis=mybir.AxisListType.X, op=ADD)
                    nc.vector.reciprocal(r2[:, par], s2[:, par])
                    with nc.allow_low_precision(reason="w is fp8 by design"):
                        nc.gpsimd.tensor_mul(
                            w2[:, par], e2[:, par],
                            r2[:, par, :, None].to_broadcast(
                                [P, HEADS, SLICE_NUM]))
                if prev is not None:
                    pool_and_transpose(*prev)
                prev = (g, w2, xnt)
            pool_and_transpose(*prev)
            emit_cc()

        # ---------------- tiny slice attention ----------------
        with tc.tile_pool(name="mps", bufs=1, space="PSUM") as mps, \
             tc.tile_pool(name="msb", bufs=2) as msb:
            for q4 in range(4):
                norm = pooled_sb[:, q4, DIM:DIM + 1]
                nrm = msb.tile([P, 1], F32)
                nc.vector.tensor_scalar_add(nrm[:], norm, 1e-5)
                rho = msb.tile([P, 1], F32)
                nc.vector.reciprocal(rho[:], nrm[:])
                pn = msb.tile([P, DIM], F32)
                nc.vector.tensor_scalar_mul(pn[:], pooled_sb[:, q4, 0:DIM],
                                            rho[:])
                pxT_p = mps.tile([P, 2, P], F32, name="pxT_p")
                for half in range(2):
                    nc.tensor.transpose(pxT_p[:, half, :],
                                        pn[:, half * P:(half + 1) * P],
                                        idf_sb[:])
                pxT = msb.tile([P, 2, P], F32)
                nc.scalar.copy(pxT[:], pxT_p[:])
                # stT[c, j*64+g] directly: lhsT = wfx (per head/half)
                stT_p = mps.tile([64, 2, 64], F32, name="stT_p")
                for j in range(2):
                    for half in range(2):
                        nc.tensor.matmul(
                            stT_p[:, j, :],
                            wfxT_sb[:, half, 2 * q4 + j, :],
                            pxT[:, half, j * 64:(j + 1) * 64],
                            start=(half == 0), stop=(half == 1))
                stT = msb.tile([64, 2, 64], F32)
                nc.vector.tensor_add(
                    stT[:], stT_p[:],
                    bfxT_sb[:, q4, :, None].to_broadcast([64, 2, 64]))
                stTv = stT[:, :, :]  # [64, 128] view via free dims (2,64)
                qk_p = mps.tile([64, 2, P], F32, name="qk_p")
                nc.tensor.matmul(qk_p[:, 0, :], wq_sb[:], stTv,
                                 start=True, stop=True)
                nc.tensor.matmul(qk_p[:, 1, :], wk_sb[:], stTv,
                                 start=True, stop=True)
                qk = msb.tile([64, 2, P], F32)
                nc.vector.tensor_copy(qk[:], qk_p[:])
                L_p = mps.tile([P, 64], F32, name="L_p")
                for j in range(2):
                    nc.tensor.matmul(L_p[j * 64:(j + 1) * 64, :],
                                     qk[:, 0, j * 64:(j + 1) * 64],
                                     qk[:, 1, j * 64:(j + 1) * 64],
                                     start=True, stop=True)
                ea = msb.tile([P, 64], F32)
                srow = msb.tile([P, 1], F32)
                nc.scalar.activation(ea[:], L_p[:], EXPF, accum_out=srow[:])
                rha = msb.tile([P, 1], F32)
                nc.vector.reciprocal(rha[:], srow[:])
                attn = msb.tile([P, 64], F32)
                nc.vector.tensor_scalar_mul(attn[:], ea[:], rha[:])
                aT_p = mps.tile([64, P], F32, name="aT_p")
                nc.tensor.transpose(aT_p[:], attn[:], idf_sb[:])
                aT = msb.tile([64, P], F32)
                nc.scalar.copy(aT[:], aT_p[:])
                osT_p = mps.tile([64, 2, 64], F32, name="osT_p")
                for j in range(2):
                    v_p = mps.tile([64, 64], F32, name="v_p")
                    nc.tensor.matmul(v_p[:], stT[:, j, :], wv_sb[:],
                                     start=True, stop=True)
                    v_sb = msb.tile([64, 64], F32)
                    nc.vector.tensor_copy(v_sb[:], v_p[:])
                    nc.tensor.matmul(osT_p[:, j, :], v_sb[:],
                                     aT[:, j * 64:(j + 1) * 64],
                                     start=True, stop=True)
                osT = msb.tile([64, 2, 64], F32)
                nc.vector.tensor_copy(osT[:], osT_p[:])
                m2_p = mps.tile([P, DIM], F32, name="m2_p")
                for j in range(2):
                    nc.tensor.matmul(m2_p[j * 64:(j + 1) * 64, :],
                                     osT[:, j, :], wo_sb[:, 2 * q4 + j, :],
                                     start=True, stop=True)
                with nc.allow_low_precision(reason="m2 fp8 + error feedback"):
                    nc.scalar.copy(m28a_sb[:, q4, :], m2_p[:])
                    if M2_RESID:
                        dm = msb.tile([P, DIM], F32)
                        nc.vector.tensor_tensor(dm[:], m2_p[:],
                                                m28a_sb[:, q4, :], op=SUB)
                        bq = msb.tile([P, DIM], FP8)
                        nc.vector.tensor_scalar_mul(bq[:], dm[:], 16.0)
                        nc.vector.tensor_scalar_mul(m28b_sb[:, q4, :], bq[:],
                                                    1.0 / 16.0)

        # ---------------- pass 2: unpool + output proj ----------------
        with tc.tile_pool(name="p2ps", bufs=6, space="PSUM") as p2ps, \
             tc.tile_pool(name="p2sb", bufs=6) as p2sb:
            for qt in range(NP2):
                for half in range(2):
                    op = p2ps.tile([P, P2W], F32)
                    toks = slice(qt * P2W, (qt + 1) * P2W)
                    hb = slice(half * P, (half + 1) * P)
                    nc.tensor.matmul(op[:], m28a_sb[:, 0:2, hb],
                                     wT_sb[:, 0:2, toks],
                                     start=True, stop=False, perf_mode=DR)
                    nc.tensor.matmul(op[:], m28a_sb[:, 2:4, hb],
                                     wT_sb[:, 2:4, toks],
                                     start=False, stop=not M2_RESID,
                                     perf_mode=DR)
                    if M2_RESID:
                        nc.tensor.matmul(op[:], m28b_sb[:, 0:2, hb],
                                         wT_sb[:, 0:2, toks],
                                         start=False, stop=False,
                                         perf_mode=DR)
                        nc.tensor.matmul(op[:], m28b_sb[:, 2:4, hb],
                                         wT_sb[:, 2:4, toks],
                                         start=False, stop=True,
                                         perf_mode=DR)
                    ob = p2sb.tile([P, P2W], BF16)
                    if (qt * 2 + half) % 2 == 0:
                        nc.vector.tensor_scalar_mul(ob[:], op[:], 1.0 / S_M2)
                    else:
                        nc.scalar.mul(ob[:], op[:], 1.0 / S_M2)
                    nc.sync.dma_start(out_ap[hb, toks], ob[:])
        if dbg:
            nc.sync.dma_start(dbg_pooled[:], pooled_sb[:])
            nc.sync.dma_start(dbg_m2a[:], m28a_sb[:])
            nc.sync.dma_start(dbg_m2b[:], m28b_sb[:])
            nc.sync.dma_start(dbg_wT[:], wT_sb[:])
    nc.compile()
    return nc


def prep_weights(inputs):
    f32 = np.float32
    Wfx = np.asarray(inputs["Wfx"], f32)
    bfx = np.asarray(inputs["bfx"], f32)
    Wx = np.asarray(inputs["Wx"], f32)
    bx = np.asarray(inputs["bx"], f32)
    Wslice = np.asarray(inputs["Wslice"], f32)
    bslice = np.asarray(inputs["bslice"], f32)
    tau = np.asarray(inputs["temperature"], f32).reshape(HEADS)
    Wq = np.asarray(inputs["Wq"], f32)
    Wk = np.asarray(inputs["Wk"], f32)
    Wv = np.asarray(inputs["Wv"], f32)
    Wout = np.asarray(inputs["Wout"], f32)
    bout = np.asarray(inputs["bout"], f32)

    wlg_blocks = []
    blg_blocks = []
    for h in range(HEADS):
        Wx_h = Wx[h * DIM_HEAD:(h + 1) * DIM_HEAD, :]
        bx_h = bx[h * DIM_HEAD:(h + 1) * DIM_HEAD]
        wlg_blocks.append((Wslice @ Wx_h) / tau[h])
        blg_blocks.append((Wslice @ bx_h + bslice) / tau[h])
    wlgT = np.ascontiguousarray(np.concatenate(wlg_blocks, 0).T, f32)
    blg = np.concatenate(blg_blocks, 0).astype(f32)

    def f8(a):
        return np.asarray(a, FP8_NP)

    def f8f(a):
        return f8(a).astype(f32)

    w8 = f8(wlgT)                       # [256, 512]
    q0 = f8(blg)
    q1 = f8(16.0 * (blg - q0.astype(f32)))
    # only rows 254/255 are read (bias8 lhsT is zero elsewhere)
    w816b = np.zeros_like(w8)
    w816b[254, :] = q0
    w816b[255, :] = q1

    # stT matmul weights: wfxT[d', half, h, c] = Wfx[h*64+c, half*128+d']
    wfxT = np.ascontiguousarray(
        Wfx.reshape(HEADS, 64, 2, P).transpose(3, 2, 0, 1), f32)
    # bfxT[c, q4, j] = bfx[(2*q4+j)*64 + c]
    bfxT = np.ascontiguousarray(
        bfx.reshape(4, 2, 64).transpose(2, 0, 1), f32)

    id8x = np.zeros((P, 2, 2 * P), FP8_NP)
    for k in range(P):
        id8x[k, 0, k] = 1.0
        id8x[k, 1, P + k] = 1.0
    bias8 = np.zeros((P, 2, P), FP8_NP)
    bias8[126, 1, :] = 1.0
    bias8[127, 1, :] = 1.0 / 16.0

    scale = DIM_HEAD ** -0.5
    return {
        "w8T": np.ascontiguousarray(w8),
        "w816T": np.ascontiguousarray(w816b),
        "id8x": id8x,
        "bias8": bias8,
        "wfxT": wfxT,
        "bfxT": bfxT,
        "wqT": np.ascontiguousarray((Wq * scale).T, f32),
        "wkT": np.ascontiguousarray(Wk.T, f32),
        "wvT": np.ascontiguousarray(Wv.T, f32),
        "woT": np.ascontiguousarray(
            Wout.T.reshape(HEADS, DIM_HEAD, DIM).transpose(1, 0, 2),
            f32) * S_M2,
        "idf32": np.eye(P, dtype=f32),
    }


_PROG = {}


def _get_prog(nshard, dbg=False):
    if (nshard, dbg) not in _PROG:
        _PROG[(nshard, dbg)] = build_program(nshard, dbg)
    return _PROG[(nshard, dbg)]


def run(inputs, nshard=NSHARD, trace=False, trace_cores=None, dbg=False):
    x = np.asarray(inputs["x"], np.float32)
    b_, n_, d_ = x.shape
    assert d_ == DIM and n_ == 2 * nshard and b_ == B
    nc = _get_prog(nshard, dbg)
    common = prep_weights(inputs)
    in_maps = []
    for core in range(NCORES):
        bb, half = core // 2, core % 2
        xs = x[bb, half * nshard:(half + 1) * nshard, :]  # [nshard, 256]
        x8 = np.asarray(xs, FP8_NP)
        m = dict(common)
        m["x8N"] = np.ascontiguousarray(x8)
        m["x8T"] = np.ascontiguousarray(x8.T)
        in_maps.append(m)
    res = run_bass_kernel_spmd(nc, in_maps, list(range(NCORES)),
                               trace=trace, trace_cores=trace_cores)
    full = np.empty((B, n_, DIM), np.float32)
    for core in range(NCORES):
        bb, half = core // 2, core % 2
        full[bb, half * nshard:(half + 1) * nshard, :] = \
            res.results[core]["outT"].T.astype(np.float32)
    full += np.asarray(inputs["bout"], np.float32)[None, None, :]
    return full, res


def kernel(**inputs):
    out, _ = run(inputs)
    return out
